# revision 1
# baseline (speedup 1.0000x reference)
"""GAT (2-layer, PyG-style) Trainium2 kernel — 8-core SPMD.

Contract: kernel(**inputs) takes FULL inputs (as produced by the problem's
setup_inputs()) and returns the FULL [N, 64] float32 output.

Strategy (dst-sharded message passing):
  - nodes partitioned into 8 contiguous shards (12500 per core); every edge is
    owned by the core that owns its dst node.  Each core sees a ROTATED node
    numbering (own shard first) so all addressing is SPMD-static.
  - Phase A (replicated): each core computes table1[n] = [h=x@W1 | a_src | a_dst]
    (bf16, 264 used cols, 768B row stride) for ALL nodes into its own HBM.
    Attention logits come free as 4 extra matmul columns (W1 is augmented).
  - Phase B: per 128-dst tile, dma_gather of table1 rows for the tile's edges
    (src rows, split into 4 int16-addressable table quarters) plus a 4B gather
    of a_dst per edge; w = exp(leaky_relu(a_s+a_d)) per edge; h rows scaled by
    w in place; one-hot [edge, dst-slot] matrix via is_equal against iota; the
    TensorEngine matmul then performs the segment softmax reduction (numerator
    and denominator in one PSUM accumulation).  Epilogue: normalize, mean
    heads, bias, relu, then the layer-2 table rows [h2 | a_s2 | a_d2].
  - AllGather of the layer-2 table shards across the 8 cores.
  - Phase C: same machinery for layer 2 -> output shard.
"""

import sys

for _p in ("/opt/trn_rl_repo",):
    if _p not in sys.path:
        sys.path.insert(0, _p)

import numpy as np

from concourse import ap_utils, bacc, bass, mybir
from concourse import tile
from concourse.bass import MemorySpace, exact_div, round_up_to_multiple
from concourse.bass_utils import run_bass_kernel_spmd

BF16 = mybir.dt.bfloat16
F32 = mybir.dt.float32
I16 = mybir.dt.int16
NP_BF16 = mybir.dt.np(BF16)

# ---------------------------------------------------------------- problem dims
N = 100000
E = 1600000
IN_DIM, HIDDEN, OUT_DIM, HEADS = 256, 128, 64, 2
NEG_SLOPE = 0.2
C1 = HEADS * HIDDEN  # 256

FULL_CFG = dict(n_cores=8, shard=12500, grp=3)

P = 128
NQ = 4                      # table quarters (int16 index range)
T1C = 264                   # table1 used cols: 256 h | 2 a_s | 2 a_d
T1S = 384                   # table1 row stride in elements (768B, mult of 256B)
T2C = 66                    # table2 used cols: 64 h2 | 1 a_s2 | 1 a_d2
T2S = 128                   # table2 row stride in elements (256B)


# ================================================================ gather
def _dma_gather(gp, out_ap, in_ap, idxs_ap, num_idxs, elem_size, elem_step):
    """bass.dma_gather with the elem%256B assert relaxed (ucode handles any
    elem size; only the row stride must be a multiple of 256B) and
    single_packet disabled (coalescing breaks past ~1k descriptors)."""
    assert idxs_ap.dtype == mybir.dt.int16
    assert in_ap.dtype == out_ap.dtype
    elem_size_bytes = elem_size * mybir.dt.size(in_ap.dtype)
    assert elem_size_bytes > 0 and elem_size_bytes % 4 == 0
    assert in_ap.space == MemorySpace.DRAM
    assert idxs_ap.space == MemorySpace.SBUF and out_ap.space == MemorySpace.SBUF
    assert ap_utils.ap_is_contiguous(out_ap.ap[1:])
    assert ap_utils.ap_is_contiguous(idxs_ap.ap[1:])
    assert in_ap.ap[-1][1] == elem_size
    assert out_ap.ap[-1][1] == elem_size
    assert out_ap.ap[0][1] * out_ap.ap[1][1] == round_up_to_multiple(num_idxs, 128)
    assert in_ap.ap[0][0] == elem_step
    stride_bytes = elem_step * mybir.dt.size(in_ap.dtype)
    stride_bytes_256 = exact_div(stride_bytes, 256)
    assert 0 < stride_bytes_256 < 256
    _in_ap = gp.lower_ap_dma(in_ap, for_custom_bir_dma=True)
    return gp.add_instruction(mybir.InstDMAGatherAnt(
        name=gp.bass.get_next_instruction_name(),
        ins=[*_in_ap, gp.lower_ap(idxs_ap),
             gp.lower_val_access(gp.to_reg(num_idxs))],
        outs=[gp.lower_ap(out_ap)],
        transpose=False, num_idxs=num_idxs, elem_size=elem_size,
        stride_bytes_256=stride_bytes_256, gen_mode=0, single_packet=False,
        queue_num=0, sbuf_tokens_per_rank=0, sbuf_free_dim_per_rank=0,
        sbuf_free_dim_pad_per_rank=0, sbuf_byte_offset=0))


# ================================================================ host prep
def _snake16(flat):
    """int16 index layout for dma_gather: logical index k sits at
    [partition k%16 (replicated x8), column k//16]."""
    cols = len(flat) // 16
    return np.tile(flat.reshape(cols, 16).T, (8, 1))


def _pack_layer(src_q, src_r, dst_local, n_tiles):
    """Group this core's edges into (tile, quarter) cells, sorted by src row
    within a cell.

    src_q: quarter of each edge's src row; src_r: row within quarter;
    dst_local: local dst id (0..shard).
    Returns (src rows, dst slots, dst locals) in packed order plus per-cell
    counts and start offsets.
    """
    t_c = dst_local >> 7
    slot = (dst_local & 127).astype(np.float32)
    order = np.lexsort((src_r, src_q, t_c))
    cell = (t_c * NQ + src_q)[order]
    sr = src_r[order]
    sl = slot[order]
    dls = dst_local[order].astype(np.int16)
    counts = np.bincount(cell, minlength=n_tiles * NQ).reshape(n_tiles, NQ)
    starts = np.zeros(n_tiles * NQ + 1, dtype=np.int64)
    np.cumsum(counts.reshape(-1), out=starts[1:])
    return sr, sl, counts, starts, dls


def _build_streams(per_core, n_tiles, grp, qch):
    """Build the snake16 src-index stream, snake16 dst-index stream and the
    plain dstloc stream for one layer, given per-core packed cells."""
    n_cores = len(per_core)
    ch = NQ * qch
    groups = [(g, min(grp, n_tiles - g)) for g in range(0, n_tiles, grp)]
    scols = sum(gn * qch * 8 * NQ for _, gn in groups)
    dcols = sum(gn * ch * 8 for _, gn in groups)
    lcols = n_tiles * ch
    srcq16 = np.zeros((n_cores, P, scols), dtype=np.int16)
    dstl16 = np.zeros((n_cores, P, dcols), dtype=np.int16)
    dstloc = np.full((n_cores, P, lcols), 255.0, dtype=NP_BF16)
    for c, (sr, sl, counts, starts, dloc_sorted) in enumerate(per_core):
        scol = dcol = 0
        for g0, gn in groups:
            ni_q = gn * qch * P
            # src stream: per quarter, tiles' cells padded to qch*128
            for q in range(NQ):
                flat = np.zeros(ni_q, dtype=np.int16)
                for ti in range(gn):
                    t = g0 + ti
                    s0 = starts[t * NQ + q]
                    cnt = counts[t, q]
                    base = ti * qch * P
                    flat[base:base + cnt] = sr[s0:s0 + cnt]
                srcq16[c, :, scol:scol + ni_q // 16] = _snake16(flat)
                scol += ni_q // 16
            # dst stream + dstloc: (q, t, j) chunk order
            ni_d = gn * ch * P
            flatd = np.zeros(ni_d, dtype=np.int16)
            flatl = np.full(ni_d, 255.0, dtype=np.float32)
            for q in range(NQ):
                for ti in range(gn):
                    t = g0 + ti
                    s0 = starts[t * NQ + q]
                    cnt = counts[t, q]
                    base = ((q * gn) + ti) * qch * P
                    flatd[base:base + cnt] = dloc_sorted[s0:s0 + cnt]
                    flatl[base:base + cnt] = sl[s0:s0 + cnt]
            dstl16[c, :, dcol:dcol + ni_d // 16] = _snake16(flatd)
            gbase = g0 * ch
            dstloc[c, :, gbase:gbase + gn * ch] = (
                flatl.reshape(gn * ch, P).T.astype(NP_BF16))
            dcol += ni_d // 16
    return srcq16, dstl16, dstloc


def _host_inputs(x, edge_index, W1, att_src1, att_dst1, b1, W2, att_src2,
                 att_dst2, b2, cfg):
    n_cores, shard, grp = cfg["n_cores"], cfg["shard"], cfg["grp"]
    n = x.shape[0]
    npad = ((n + 511) // 512) * 512
    assert npad % NQ == 0 and n % NQ == 0
    qs1, qs2 = npad // NQ, n // NQ
    assert qs1 <= 32768 and qs2 <= 32768 and shard <= 32768
    n_tiles = (shard + P - 1) // P

    loop = np.arange(n, dtype=np.int64)
    src = np.concatenate([np.asarray(edge_index[0]), loop]).astype(np.int64)
    dst = np.concatenate([np.asarray(edge_index[1]), loop]).astype(np.int64)
    core_of = dst // shard

    per_core_1, per_core_2 = [], []
    maxq1 = maxq2 = 0
    for c in range(n_cores):
        sel = core_of == c
        s_c, d_c = src[sel], dst[sel]
        dl = (d_c - c * shard).astype(np.int64)
        rot = (s_c - c * shard) % n
        # layer 1 (rotated ids)
        pc1 = _pack_layer(rot // qs1, (rot % qs1).astype(np.int16), dl, n_tiles)
        per_core_1.append(pc1)
        maxq1 = max(maxq1, int(pc1[2].max()))
        # layer 2 (global ids)
        pc2 = _pack_layer(s_c // qs2, (s_c % qs2).astype(np.int16), dl, n_tiles)
        per_core_2.append(pc2)
        maxq2 = max(maxq2, int(pc2[2].max()))

    qch1 = max(1, (maxq1 + P - 1) // P)
    qch2 = max(1, (maxq2 + P - 1) // P)
    s1, d1, l1 = _build_streams(per_core_1, n_tiles, grp, qch1)
    s2, d2, l2 = _build_streams(per_core_2, n_tiles, grp, qch2)

    x = np.asarray(x, dtype=np.float32)
    W1 = np.asarray(W1, dtype=np.float32)
    a_s1 = np.asarray(att_src1, dtype=np.float32)
    a_d1 = np.asarray(att_dst1, dtype=np.float32)
    w_as = np.einsum("khc,hc->kh", W1.reshape(IN_DIM, HEADS, HIDDEN), a_s1)
    w_ad = np.einsum("khc,hc->kh", W1.reshape(IN_DIM, HEADS, HIDDEN), a_d1)
    W1aug = np.concatenate([W1, w_as, w_ad], axis=1).astype(NP_BF16)  # [256,260]

    W2 = np.asarray(W2, dtype=np.float32)
    a_s2 = np.asarray(att_src2, dtype=np.float32).reshape(OUT_DIM)
    a_d2 = np.asarray(att_dst2, dtype=np.float32).reshape(OUT_DIM)
    W2aug = np.concatenate(
        [W2, (W2 @ a_s2)[:, None], (W2 @ a_d2)[:, None]], axis=1
    ).astype(NP_BF16)                              # [128, 66]

    b1F = np.tile(np.asarray(b1, dtype=np.float32)[None, :], (P, 1))
    b2F = np.tile(np.asarray(b2, dtype=np.float32)[None, :], (P, 1))
    iotaF = np.tile(np.arange(P, dtype=np.float32)[None, :], (P, 1)).astype(NP_BF16)
    identT = np.eye(P, dtype=np.float32).astype(NP_BF16)

    shared = dict(W1aug=W1aug, W2aug=W2aug, b1F=b1F, b2F=b2F, iotaF=iotaF,
                  identT=identT)
    in_maps = []
    for c in range(n_cores):
        xr = np.roll(x, -c * shard, axis=0)
        xT = np.zeros((IN_DIM, npad), dtype=NP_BF16)
        xT[:, :n] = xr.T.astype(NP_BF16)
        m = dict(shared)
        m["xT"] = xT
        m["srcq1"], m["dstl1"], m["dloc1"] = s1[c], d1[c], l1[c]
        m["srcq2"], m["dstl2"], m["dloc2"] = s2[c], d2[c], l2[c]
        in_maps.append(m)
    meta = dict(qch1=qch1, qch2=qch2, npad=npad, maxq1=maxq1, maxq2=maxq2)
    return in_maps, meta


# ================================================================ device prog
def build_program(cfg, meta):
    n_cores, shard, grp = cfg["n_cores"], cfg["shard"], cfg["grp"]
    n = cfg.get("n", N)
    npad = meta["npad"]
    qch1, qch2 = meta["qch1"], meta["qch2"]
    qs1, qs2 = npad // NQ, n // NQ
    n_tiles = (shard + P - 1) // P
    last_rows = shard - (n_tiles - 1) * P
    groups = [(g, min(grp, n_tiles - g)) for g in range(0, n_tiles, grp)]
    stop_after = cfg.get("stop_after")

    nc = bacc.Bacc("TRN2", target_bir_lowering=False, debug=False,
                   num_devices=n_cores)

    def din(name, shape, dt):
        return nc.dram_tensor(name, shape, dt, kind="ExternalInput").ap()

    xT = din("xT", [IN_DIM, npad], BF16)
    W1aug = din("W1aug", [IN_DIM, C1 + 4], BF16)
    W2aug = din("W2aug", [HIDDEN, OUT_DIM + 2], BF16)
    b1F = din("b1F", [P, HIDDEN], F32)
    b2F = din("b2F", [P, OUT_DIM], F32)
    iotaF = din("iotaF", [P, P], BF16)
    identT = din("identT", [P, P], BF16)
    scols1 = sum(gn * qch1 * 8 * NQ for _, gn in groups)
    scols2 = sum(gn * qch2 * 8 * NQ for _, gn in groups)
    srcq1 = din("srcq1", [P, scols1], I16)
    dstl1 = din("dstl1", [P, scols1], I16)
    dloc1 = din("dloc1", [P, n_tiles * NQ * qch1], BF16)
    srcq2 = din("srcq2", [P, scols2], I16)
    dstl2 = din("dstl2", [P, scols2], I16)
    dloc2 = din("dloc2", [P, n_tiles * NQ * qch2], BF16)
    out_shard = nc.dram_tensor("out_shard", [shard, OUT_DIM], F32,
                               kind="ExternalOutput").ap()

    with tile.TileContext(nc) as tc:
        with (
            tc.tile_pool(name="dram", bufs=1, space="DRAM") as dram,
            tc.tile_pool(name="const", bufs=1) as cpool,
        ):
            table1 = dram.tile([npad, T1S], BF16)
            t2shard = dram.tile([shard, T2S], BF16)
            t2full = dram.tile([shard * n_cores, T2S], BF16,
                               addr_space="Shared" if n_cores > 4 else "Local")

            w1a = cpool.tile([P, C1 + 4], BF16, tag="w1a")
            w1b = cpool.tile([P, C1 + 4], BF16, tag="w1b")
            nc.sync.dma_start(out=w1a[:, :], in_=W1aug[0:P, :])
            nc.sync.dma_start(out=w1b[:, :], in_=W1aug[P:2 * P, :])
            w2_sb = cpool.tile([P, OUT_DIM + 2], BF16, tag="w2")
            nc.sync.dma_start(out=w2_sb[:, :], in_=W2aug[:, :])
            b1_sb = cpool.tile([P, HIDDEN], F32, tag="b1")
            nc.sync.dma_start(out=b1_sb[:, :], in_=b1F[:, :])
            b2_sb = cpool.tile([P, OUT_DIM], F32, tag="b2")
            nc.sync.dma_start(out=b2_sb[:, :], in_=b2F[:, :])
            iota_sb = cpool.tile([P, P], BF16, tag="iota")
            nc.sync.dma_start(out=iota_sb[:, :], in_=iotaF[:, :])
            id_sb = cpool.tile([P, P], BF16, tag="ident")
            nc.sync.dma_start(out=id_sb[:, :], in_=identT[:, :])

            # ---------------- Phase A: table1 = [x@W1aug] for all nodes
            SLAB = 512
            with (
                tc.tile_pool(name="pa_sbuf", bufs=3) as pa,
                tc.tile_pool(name="pa_lhs", bufs=2) as pl,
                tc.tile_pool(name="pa_psum", bufs=4, space="PSUM") as pp,
            ):
                for s in range(npad // SLAB):
                    lhs0 = pl.tile([P, SLAB], BF16, tag="lhs0")
                    lhs1 = pl.tile([P, SLAB], BF16, tag="lhs1")
                    nc.sync.dma_start(out=lhs0[:, :],
                                      in_=xT[0:P, s * SLAB:(s + 1) * SLAB])
                    nc.sync.dma_start(out=lhs1[:, :],
                                      in_=xT[P:2 * P, s * SLAB:(s + 1) * SLAB])
                    for ci in range(SLAB // P):
                        rbase = s * SLAB + ci * P
                        ps = pp.tile([P, C1 + 4], F32, tag="pa_ps", space="PSUM")
                        nc.tensor.matmul(ps[:, :],
                                         lhsT=lhs0[:, ci * P:(ci + 1) * P],
                                         rhs=w1a[:, :], start=True, stop=False)
                        nc.tensor.matmul(ps[:, :],
                                         lhsT=lhs1[:, ci * P:(ci + 1) * P],
                                         rhs=w1b[:, :], start=False, stop=True)
                        stage = pa.tile([P, C1 + 4], BF16, tag="pa_stage")
                        if ci % 2 == 0:
                            nc.vector.tensor_copy(out=stage[:, :], in_=ps[:, :])
                        else:
                            nc.scalar.copy(out=stage[:, :], in_=ps[:, :])
                        nc.sync.dma_start(
                            out=table1[rbase:rbase + P, 0:C1 + 4],
                            in_=stage[:, :])

            if stop_after != "A":
                _gat_layer(
                    nc, tc, layer=1, groups=groups, qch=qch1, n_tiles=n_tiles,
                    last_rows=last_rows, table=table1, tab_step=T1S,
                    qsize=qs1, hdim=C1, heads=HEADS,
                    srcq=srcq1, dstl=dstl1, dlocT=dloc1,
                    iota_sb=iota_sb, id_sb=id_sb, w2_sb=w2_sb,
                    b1_sb=b1_sb, b2_sb=b2_sb, adtab=table1, ad_step=T1S,
                    ad_lo=C1 + 2, t2shard=t2shard, out_shard=out_shard,
                    debug_out=(stop_after == "B"),
                )

            if stop_after is None:
                nc.gpsimd.collective_compute(
                    "AllGather", mybir.AluOpType.bypass,
                    replica_groups=[list(range(n_cores))],
                    ins=[t2shard[:, :]],
                    outs=[t2full[:, :]],
                )
                _gat_layer(
                    nc, tc, layer=2, groups=groups, qch=qch2, n_tiles=n_tiles,
                    last_rows=last_rows, table=t2full, tab_step=T2S,
                    qsize=qs2, hdim=OUT_DIM, heads=1,
                    srcq=srcq2, dstl=dstl2, dlocT=dloc2,
                    iota_sb=iota_sb, id_sb=id_sb, w2_sb=w2_sb,
                    b1_sb=b1_sb, b2_sb=b2_sb, adtab=t2shard, ad_step=T2S,
                    ad_lo=OUT_DIM, t2shard=t2shard, out_shard=out_shard,
                )
            elif stop_after == "A":
                with tc.tile_pool(name="dbgA", bufs=2) as pd:
                    for t in range(min(shard, 1024) // P):
                        st = pd.tile([P, T1C], BF16, tag="dbgA_t")
                        nc.sync.dma_start(out=st[:, :],
                                          in_=table1[t * P:(t + 1) * P, 0:T1C])
                        sf = pd.tile([P, OUT_DIM], F32, tag="dbgA_f")
                        nc.vector.tensor_copy(out=sf[:, :], in_=st[:, 0:OUT_DIM])
                        nc.sync.dma_start(out=out_shard[t * P:(t + 1) * P, :],
                                          in_=sf[:, :])

    nc.finalize()
    return nc


def _gat_layer(nc, tc, *, layer, groups, qch, n_tiles, last_rows, table,
               tab_step, qsize, hdim, heads, srcq, dstl, dlocT, iota_sb,
               id_sb, w2_sb, b1_sb, b2_sb, adtab, ad_step, ad_lo, t2shard,
               out_shard, debug_out=False):
    as_off = hdim            # a_s column(s) in the gathered row
    tcols = hdim + 2 * heads  # gathered row: h | a_s | a_d
    rcols = hdim + heads     # matmul rhs cols (h plus per-head w)
    ch = NQ * qch
    name = f"L{layer}"
    scol = dcol = 0
    with (
        tc.tile_pool(name=f"{name}_gath", bufs=2) as pg,
        tc.tile_pool(name=f"{name}_m", bufs=2) as pm,
        tc.tile_pool(name=f"{name}_sm", bufs=3) as psm,
        tc.tile_pool(name=f"{name}_idx", bufs=2) as pidx,
        tc.tile_pool(name=f"{name}_psum", bufs=2, space="PSUM") as pps,
    ):
        for g0, gn in groups:
            ni_q = gn * qch * P
            ni_d = gn * ch * P
            sidx = pidx.tile([P, NQ * ni_q // 16], I16, tag="sidx")
            didx = pidx.tile([P, ni_d // 16], I16, tag="didx")
            dloc = pidx.tile([P, gn * ch], BF16, tag="dloc")
            nc.sync.dma_start(out=sidx[:, :],
                              in_=srcq[:, scol:scol + NQ * ni_q // 16])
            nc.sync.dma_start(out=didx[:, :],
                              in_=dstl[:, dcol:dcol + ni_d // 16])
            nc.sync.dma_start(out=dloc[:, :],
                              in_=dlocT[:, g0 * ch:(g0 + gn) * ch])
            scol += NQ * ni_q // 16
            dcol += ni_d // 16

            hg = pg.tile([P, NQ, gn, qch, tcols], BF16, tag="hg")
            for q in range(NQ):
                _dma_gather(
                    nc.gpsimd,
                    hg[:, q, :, :, :].rearrange("p a b c -> p (a b) c"),
                    table[q * qsize:(q + 1) * qsize, 0:tcols],
                    sidx[:, q * ni_q // 16:(q + 1) * ni_q // 16],
                    ni_q, tcols, tab_step)
            adg = pg.tile([P, NQ, gn, qch, 2], BF16, tag="adg")
            _dma_gather(
                nc.gpsimd,
                adg[:, :, :, :, :].rearrange("p q a b c -> p (q a b) c"),
                adtab[:, ad_lo:ad_lo + 2],
                didx[:, :], ni_d, 2, ad_step)

            for ti in range(gn):
                tid = g0 + ti
                rows = last_rows if tid == n_tiles - 1 else P
                ht = hg[:, :, ti, :, :]          # [P, NQ, qch, tcols]
                adt = adg[:, :, ti, :, 2 - heads:2]  # [P, NQ, qch, heads]
                dlt = dloc[:, :].rearrange(
                    "p (q a b) -> p q a b", q=NQ, a=gn)[:, :, ti, :]

                sE = psm.tile([P, NQ, qch, heads], F32, tag="sE")
                nc.vector.tensor_tensor(
                    out=sE[:, :, :, :], in0=ht[:, :, :, as_off:as_off + heads],
                    in1=adt, op=mybir.AluOpType.add)
                lrE = psm.tile([P, NQ, qch, heads], F32, tag="lrE")
                nc.vector.scalar_tensor_tensor(
                    out=lrE[:, :, :, :], in0=sE[:, :, :, :], scalar=NEG_SLOPE,
                    in1=sE[:, :, :, :], op0=mybir.AluOpType.mult,
                    op1=mybir.AluOpType.max)
                nc.scalar.activation(
                    out=ht[:, :, :, as_off:as_off + heads], in_=lrE[:, :, :, :],
                    func=mybir.ActivationFunctionType.Exp)

                for q in range(NQ):
                    hv = ht[:, q, :, 0:hdim].rearrange(
                        "p a (h c) -> p a h c", h=heads)
                    wv = ht[:, q, :, as_off:as_off + heads].to_broadcast(
                        [P, qch, heads, hdim // heads])
                    nc.vector.tensor_tensor(out=hv, in0=hv, in1=wv,
                                            op=mybir.AluOpType.mult)

                mt = pm.tile([P, ch, P], BF16, tag="mt")
                iota_ap = iota_sb[:, :]
                iota_v = bass.AP(
                    iota_ap.tensor, iota_ap.offset,
                    [list(iota_ap.ap[0]), [0, NQ], [0, qch], [1, P]])
                nc.vector.tensor_tensor(
                    out=mt[:, :, :].rearrange("p (q a) b -> p q a b", q=NQ),
                    in0=dlt.to_broadcast([P, NQ, qch, P]),
                    in1=iota_v, op=mybir.AluOpType.is_equal)

                ps = pps.tile([P, rcols], F32, tag="agg", space="PSUM")
                for k in range(ch):
                    nc.tensor.matmul(ps[:, :], lhsT=mt[:, k, :],
                                     rhs=ht[:, k // qch, k % qch, 0:rcols],
                                     start=(k == 0), stop=(k == ch - 1))

                rec = psm.tile([P, heads], F32, tag="rec")
                nc.vector.reciprocal(rec[:, :], ps[:, hdim:hdim + heads])

                if layer == 1:
                    t0 = psm.tile([P, HIDDEN], F32, tag="t0")
                    nc.vector.tensor_scalar(
                        out=t0[:, :], in0=ps[:, 0:HIDDEN],
                        scalar1=rec[:, 0:1], scalar2=None,
                        op0=mybir.AluOpType.mult)
                    nc.vector.scalar_tensor_tensor(
                        out=t0[:, :], in0=ps[:, HIDDEN:2 * HIDDEN],
                        scalar=rec[:, 1:2], in1=t0[:, :],
                        op0=mybir.AluOpType.mult, op1=mybir.AluOpType.add)
                    hb = psm.tile([P, HIDDEN], F32, tag="hb")
                    nc.vector.scalar_tensor_tensor(
                        out=hb[:, :], in0=t0[:, :], scalar=0.5, in1=b1_sb[:, :],
                        op0=mybir.AluOpType.mult, op1=mybir.AluOpType.add)
                    hr = psm.tile([P, HIDDEN], BF16, tag="hr")
                    nc.scalar.activation(out=hr[:, :], in_=hb[:, :],
                                         func=mybir.ActivationFunctionType.Relu)
                    psT = pps.tile([P, P], BF16, tag="psT", space="PSUM")
                    nc.tensor.transpose(out=psT[:, :], in_=hr[:, :],
                                        identity=id_sb[:, :])
                    hrT = psm.tile([P, P], BF16, tag="hrT")
                    nc.scalar.copy(out=hrT[:, :], in_=psT[:, :])
                    ps2 = pps.tile([P, OUT_DIM + 2], F32, tag="ps2",
                                   space="PSUM")
                    nc.tensor.matmul(ps2[:, :], lhsT=hrT[:, :], rhs=w2_sb[:, :],
                                     start=True, stop=True)
                    t2 = psm.tile([P, OUT_DIM + 2], BF16, tag="t2")
                    nc.vector.tensor_copy(out=t2[:, :], in_=ps2[:, :])
                    nc.sync.dma_start(
                        out=t2shard[tid * P:tid * P + rows, 0:OUT_DIM + 2],
                        in_=t2[0:rows, :])
                    if debug_out:
                        dbg = psm.tile([P, OUT_DIM], F32, tag="dbg")
                        nc.vector.tensor_copy(out=dbg[:, :],
                                              in_=ps2[:, 0:OUT_DIM])
                        nc.sync.dma_start(
                            out=out_shard[tid * P:tid * P + rows, :],
                            in_=dbg[0:rows, :])
                else:
                    of = psm.tile([P, OUT_DIM], F32, tag="of")
                    nc.vector.scalar_tensor_tensor(
                        out=of[:, :], in0=ps[:, 0:OUT_DIM], scalar=rec[:, 0:1],
                        in1=b2_sb[:, :], op0=mybir.AluOpType.mult,
                        op1=mybir.AluOpType.add)
                    nc.sync.dma_start(
                        out=out_shard[tid * P:tid * P + rows, :],
                        in_=of[0:rows, :])


# ================================================================ entry point
def kernel(**inputs):
    cfg = dict(FULL_CFG)
    cfg["n"] = N
    in_maps, meta = _host_inputs(
        inputs["x"], inputs["edge_index"], inputs["W1"], inputs["att_src1"],
        inputs["att_dst1"], inputs["b1"], inputs["W2"], inputs["att_src2"],
        inputs["att_dst2"], inputs["b2"], cfg)
    nc = build_program(cfg, meta)
    # transient device wedges (NRT_EXEC_UNIT_UNRECOVERABLE) self-heal after a
    # few minutes; retry rather than failing the whole run
    import time as _time
    last = None
    for attempt in range(4):
        try:
            res = run_bass_kernel_spmd(
                nc, in_maps, core_ids=list(range(cfg["n_cores"])))
            break
        except Exception as exc:  # noqa: BLE001
            last = exc
            if attempt == 3:
                raise
            _time.sleep(90)
    out = np.concatenate(
        [res.results[c]["out_shard"] for c in range(cfg["n_cores"])], axis=0)
    return out.astype(np.float32)



# revision 4
# speedup vs baseline: 17.7327x; 17.7327x over previous
"""GAT (2-layer, PyG-style) Trainium2 kernel — 8-core SPMD.

Contract: kernel(**inputs) takes FULL inputs (as produced by the problem's
setup_inputs()) and returns the FULL [N, 64] float32 output.

v2 vs v1: the per-edge a_d DMA-gather (half the GpSimd ucode time) is
replaced by a TensorE one-hot-transpose matmul per 128-edge chunk, and the
row gathers are de-padded (per-(group,quarter) concatenated cells with
exact chunk counts shared across cores).

Strategy (dst-sharded message passing):
  - nodes partitioned into 8 contiguous shards (12500 per core); every edge is
    owned by the core that owns its dst node.  Each core sees a ROTATED node
    numbering (own shard first) so all addressing is SPMD-static.
  - Phase A (replicated): each core computes table1[n] = [h=x@W1 | a_s | a_d]
    (bf16, 260 used cols, 768B row stride) for ALL nodes into its own HBM.
  - Phase B: per (group of tiles, src-quarter), one dma_gather pulls the
    [h | a_s] rows (258 cols) of the slab's edges.  Per 128-edge chunk,
    mtT[d, e] = onehot (built via a K=1 broadcast matmul of the host-shipped
    slot row + DVE is_equal) feeds a [128x128]x[128,2] matmul producing
    a_d[dst(e)] per edge; w = exp(leaky_relu(a_s + a_d)); h rows scaled by w;
    mt (edge-major onehot from the dloc stream) feeds the aggregation matmul
    accumulating numerator and denominator per dst tile in PSUM.
  - AllGather of the layer-2 table shards across the 8 cores; layer 2 repeats
    the machinery with heads=1.
"""

import sys

for _p in ("/opt/trn_rl_repo",):
    if _p not in sys.path:
        sys.path.insert(0, _p)

import numpy as np

from concourse import ap_utils, bacc, bass, mybir
from concourse import tile
from concourse.bass import MemorySpace, exact_div, round_up_to_multiple
from concourse.bass_utils import run_bass_kernel_spmd

BF16 = mybir.dt.bfloat16
F32 = mybir.dt.float32
I16 = mybir.dt.int16
NP_BF16 = mybir.dt.np(BF16)

# ---------------------------------------------------------------- problem dims
N = 100000
E = 1600000
IN_DIM, HIDDEN, OUT_DIM, HEADS = 256, 128, 64, 2
NEG_SLOPE = 0.2
C1 = HEADS * HIDDEN  # 256

FULL_CFG = dict(n_cores=8, shard=12500, grp=3)

P = 128
NQ = 4                      # table quarters (int16 index range)
T1S = 384                   # table1 row stride in elements (768B, mult of 256B)
G1 = C1 + 2                 # gathered cols layer 1: 256 h | 2 a_s
T2C = 66                    # table2 used cols: 64 h2 | 1 a_s2 | 1 a_d2
T2S = 128                   # table2 row stride in elements (256B)


# ================================================================ gather
def _dma_gather(gp, out_ap, in_ap, idxs_ap, num_idxs, elem_size, elem_step):
    """bass.dma_gather with the elem%256B assert relaxed (ucode handles any
    elem size; only the row stride must be a multiple of 256B) and
    single_packet disabled."""
    assert idxs_ap.dtype == mybir.dt.int16
    assert in_ap.dtype == out_ap.dtype
    elem_size_bytes = elem_size * mybir.dt.size(in_ap.dtype)
    assert elem_size_bytes > 0 and elem_size_bytes % 4 == 0
    assert in_ap.space == MemorySpace.DRAM
    assert idxs_ap.space == MemorySpace.SBUF and out_ap.space == MemorySpace.SBUF
    assert ap_utils.ap_is_contiguous(out_ap.ap[1:])
    assert ap_utils.ap_is_contiguous(idxs_ap.ap[1:])
    assert in_ap.ap[-1][1] == elem_size
    assert out_ap.ap[-1][1] == elem_size
    assert out_ap.ap[0][1] * out_ap.ap[1][1] == round_up_to_multiple(num_idxs, 128)
    assert in_ap.ap[0][0] == elem_step
    stride_bytes = elem_step * mybir.dt.size(in_ap.dtype)
    stride_bytes_256 = exact_div(stride_bytes, 256)
    assert 0 < stride_bytes_256 < 256
    _in_ap = gp.lower_ap_dma(in_ap, for_custom_bir_dma=True)
    return gp.add_instruction(mybir.InstDMAGatherAnt(
        name=gp.bass.get_next_instruction_name(),
        ins=[*_in_ap, gp.lower_ap(idxs_ap),
             gp.lower_val_access(gp.to_reg(num_idxs))],
        outs=[gp.lower_ap(out_ap)],
        transpose=False, num_idxs=num_idxs, elem_size=elem_size,
        stride_bytes_256=stride_bytes_256, gen_mode=0, single_packet=False,
        queue_num=0, sbuf_tokens_per_rank=0, sbuf_free_dim_per_rank=0,
        sbuf_free_dim_pad_per_rank=0, sbuf_byte_offset=0))


# ================================================================ host prep
def _snake16(flat):
    """int16 index layout for dma_gather: logical index k sits at
    [partition k%16 (replicated x8), column k//16]."""
    cols = len(flat) // 16
    return np.tile(flat.reshape(cols, 16).T, (8, 1))


def _pack_layer(n_cores, per_core_eq, n_tiles, grp, qs):
    """Pack one layer's edges into the shared (group, quarter) structure.

    per_core_eq: per core, tuple (src_row int, src_q int, dst_local int)
      arrays for all edges owned by the core (dst in its shard).
    Returns:
      meta_units: list over groups of list over quarters of
        dict(K=int, units=[(k, t_abs), ...])
      streams per core: idx stream (int16 snake16 concat), dlocP cols,
        dlocF array.
    """
    groups = [(g, min(grp, n_tiles - g)) for g in range(0, n_tiles, grp)]

    # per core: sort edges by (g, q, t, src) and compute per-cell counts
    core_sorted = []
    counts = np.zeros((n_cores, n_tiles, NQ), dtype=np.int64)
    for c in range(n_cores):
        sr, sq, dl = per_core_eq[c]
        t_c = dl >> 7
        order = np.lexsort((sr, t_c, sq, t_c // grp))
        core_sorted.append((sr[order], sq[order], dl[order], t_c[order]))
        np.add.at(counts[c], (t_c, sq), 1)

    meta_units = []
    idx_streams = [[] for _ in range(n_cores)]
    dlocP_cols = [[] for _ in range(n_cores)]   # list of [128] arrays
    q_unit_cols = []        # per quarter: per core list of unit cols
    for g0, gn in groups:
        g_units = []
        for q in range(NQ):
            cnts = counts[:, g0:g0 + gn, q]             # [cores, gn]
            offs = np.zeros((n_cores, gn + 1), dtype=np.int64)
            np.cumsum(cnts, axis=1, out=offs[:, 1:])
            totals = offs[:, -1]
            K = max(1, int(np.ceil(totals.max() / 128)))
            # unit set: union over cores of (k, t) with cell t touching chunk k
            unit_set = set()
            for c in range(n_cores):
                for ti in range(gn):
                    s0, s1 = offs[c, ti], offs[c, ti + 1]
                    if s1 == s0:
                        continue
                    for k in range(s0 // 128, (s1 - 1) // 128 + 1):
                        unit_set.add((int(k), ti))
            units = sorted(unit_set)
            g_units.append(dict(K=K, units=units, g0=g0, gn=gn, q=q))
            # per-core streams
            q_cols = [[] for _ in range(n_cores)]
            for c in range(n_cores):
                sr, sq, dl, t_c = core_sorted[c]
                sel = (sq == q) & (t_c >= g0) & (t_c < g0 + gn)
                srows = sr[sel]
                dls = dl[sel]
                tcs = t_c[sel]
                ni = K * 128
                flat = np.zeros(ni, dtype=np.int16)
                flat[:len(srows)] = srows
                idx_streams[c].append(flat)
                # dloc per unit
                pos = np.arange(len(srows))
                for (k, ti) in units:
                    col = np.full(128, 255.0, dtype=np.float32)
                    m = (tcs == g0 + ti) & (pos // 128 == k)
                    pp = pos[m] % 128
                    col[pp] = (dls[m] - (g0 + ti) * 128).astype(np.float32)
                    dlocP_cols[c].append(col)
                    q_cols[c].append(col)
            q_unit_cols.append(q_cols)
        meta_units.append(g_units)

    n_units = len(dlocP_cols[0])
    for c in range(n_cores):
        assert len(dlocP_cols[c]) == n_units

    # dlocP: [128, n_units] bf16
    dlocP = np.stack(
        [np.stack(cols, axis=1) for cols in dlocP_cols], axis=0
    ).astype(NP_BF16)
    # dlocF: quarter qi's unit j lives at [qi % 128, page*pcols + j*128]
    # (free-major, for the K=1 broadcast matmul); page = qi // 128.
    n_q = len(q_unit_cols)
    pcols = max(1, max(len(qc[0]) for qc in q_unit_cols)) * 128
    pages = (n_q + 127) // 128
    dlocF = np.full((n_cores, 128, pages * pcols), 255.0, dtype=NP_BF16)
    for qi, q_cols in enumerate(q_unit_cols):
        base = (qi // 128) * pcols
        for c in range(n_cores):
            for j, col in enumerate(q_cols[c]):
                dlocF[c, qi % 128, base + j * 128:base + (j + 1) * 128] = col
    # idx snake16
    sidx = np.concatenate(
        [np.concatenate([_snake16(f) for f in idx_streams[c]], axis=1)[None]
         for c in range(n_cores)], axis=0)
    return meta_units, n_units, sidx, dlocP, dlocF, pcols


def _host_inputs(x, edge_index, W1, att_src1, att_dst1, b1, W2, att_src2,
                 att_dst2, b2, cfg):
    n_cores, shard, grp = cfg["n_cores"], cfg["shard"], cfg["grp"]
    n = x.shape[0]
    npad = ((n + 511) // 512) * 512
    assert npad % NQ == 0 and n % NQ == 0
    qs1, qs2 = npad // NQ, n // NQ
    assert qs1 <= 32768 and qs2 <= 32768 and shard <= 32768
    n_tiles = (shard + P - 1) // P

    # self-loops are handled densely on-device (no gather indices)
    src = np.asarray(edge_index[0]).astype(np.int64)
    dst = np.asarray(edge_index[1]).astype(np.int64)
    core_of = dst // shard

    eq1, eq2 = [], []
    for c in range(n_cores):
        sel = core_of == c
        s_c, d_c = src[sel], dst[sel]
        dl = (d_c - c * shard).astype(np.int64)
        rot = (s_c - c * shard) % n
        eq1.append(((rot % qs1).astype(np.int16), rot // qs1, dl))
        eq2.append(((s_c % qs2).astype(np.int16), s_c // qs2, dl))

    mu1, nu1, sidx1, dlocP1, dlocF1, pc1 = _pack_layer(
        n_cores, eq1, n_tiles, grp, qs1)
    mu2, nu2, sidx2, dlocP2, dlocF2, pc2 = _pack_layer(
        n_cores, eq2, n_tiles, grp, qs2)

    x = np.asarray(x, dtype=np.float32)
    W1 = np.asarray(W1, dtype=np.float32)
    a_s1 = np.asarray(att_src1, dtype=np.float32)
    a_d1 = np.asarray(att_dst1, dtype=np.float32)
    w_as = np.einsum("khc,hc->kh", W1.reshape(IN_DIM, HEADS, HIDDEN), a_s1)
    w_ad = np.einsum("khc,hc->kh", W1.reshape(IN_DIM, HEADS, HIDDEN), a_d1)
    W1aug = np.concatenate([W1, w_as, w_ad], axis=1).astype(NP_BF16)  # [256,260]

    W2 = np.asarray(W2, dtype=np.float32)
    a_s2 = np.asarray(att_src2, dtype=np.float32).reshape(OUT_DIM)
    a_d2 = np.asarray(att_dst2, dtype=np.float32).reshape(OUT_DIM)
    W2aug = np.concatenate(
        [W2, (W2 @ a_s2)[:, None], (W2 @ a_d2)[:, None]], axis=1
    ).astype(NP_BF16)                              # [128, 66]

    b1F = np.tile(np.asarray(b1, dtype=np.float32)[None, :], (P, 1))
    b2F = np.tile(np.asarray(b2, dtype=np.float32)[None, :], (P, 1))
    iotaF = np.tile(np.arange(P, dtype=np.float32)[None, :], (P, 1)).astype(NP_BF16)
    iotaP = np.arange(P, dtype=np.float32)[:, None]          # [128, 1] f32
    ones1 = np.ones((1, P), dtype=np.float32).astype(NP_BF16)
    identT = np.eye(P, dtype=np.float32).astype(NP_BF16)

    shared = dict(W1aug=W1aug, W2aug=W2aug, b1F=b1F, b2F=b2F, iotaF=iotaF,
                  iotaP=iotaP, ones1=ones1, identT=identT)
    in_maps = []
    for c in range(n_cores):
        xr = np.roll(x, -c * shard, axis=0)
        xT = np.zeros((IN_DIM, npad), dtype=NP_BF16)
        xT[:, :n] = xr.T.astype(NP_BF16)
        m = dict(shared)
        m["xT"] = xT
        m["sidx1"], m["dlocP1"], m["dlocF1"] = sidx1[c], dlocP1[c], dlocF1[c]
        m["sidx2"], m["dlocP2"], m["dlocF2"] = sidx2[c], dlocP2[c], dlocF2[c]
        in_maps.append(m)
    meta = dict(mu1=mu1, nu1=nu1, mu2=mu2, nu2=nu2, npad=npad,
                scols1=sidx1.shape[2], scols2=sidx2.shape[2],
                fcols1=dlocF1.shape[2], fcols2=dlocF2.shape[2],
                pcols1=pc1, pcols2=pc2)
    return in_maps, meta


# ================================================================ device prog
def build_program(cfg, meta):
    n_cores, shard, grp = cfg["n_cores"], cfg["shard"], cfg["grp"]
    n = cfg.get("n", N)
    npad = meta["npad"]
    qs1, qs2 = npad // NQ, n // NQ
    n_tiles = (shard + P - 1) // P
    last_rows = shard - (n_tiles - 1) * P
    stop_after = cfg.get("stop_after")

    nc = bacc.Bacc("TRN2", target_bir_lowering=False, debug=False,
                   num_devices=n_cores)

    def din(name, shape, dt):
        return nc.dram_tensor(name, shape, dt, kind="ExternalInput").ap()

    xT = din("xT", [IN_DIM, npad], BF16)
    W1aug = din("W1aug", [IN_DIM, C1 + 4], BF16)
    W2aug = din("W2aug", [HIDDEN, OUT_DIM + 2], BF16)
    b1F = din("b1F", [P, HIDDEN], F32)
    b2F = din("b2F", [P, OUT_DIM], F32)
    iotaF = din("iotaF", [P, P], BF16)
    iotaP = din("iotaP", [P, 1], F32)
    ones1 = din("ones1", [1, P], BF16)
    identT = din("identT", [P, P], BF16)
    sidx1 = din("sidx1", [P, meta["scols1"]], I16)
    dlocP1 = din("dlocP1", [P, meta["nu1"]], BF16)
    dlocF1 = din("dlocF1", [P, meta["fcols1"]], BF16)
    sidx2 = din("sidx2", [P, meta["scols2"]], I16)
    dlocP2 = din("dlocP2", [P, meta["nu2"]], BF16)
    dlocF2 = din("dlocF2", [P, meta["fcols2"]], BF16)
    out_shard = nc.dram_tensor("out_shard", [shard, OUT_DIM], F32,
                               kind="ExternalOutput").ap()

    with tile.TileContext(nc) as tc:
        with (
            tc.tile_pool(name="dram", bufs=1, space="DRAM") as dram,
            tc.tile_pool(name="const", bufs=1) as cpool,
        ):
            table1 = dram.tile([npad, T1S], BF16)
            t2shard = dram.tile([shard, T2S], BF16)
            t2full = dram.tile([shard * n_cores, T2S], BF16,
                               addr_space="Shared" if n_cores > 4 else "Local")

            w1a = cpool.tile([P, C1 + 4], BF16, tag="w1a")
            w1b = cpool.tile([P, C1 + 4], BF16, tag="w1b")
            nc.sync.dma_start(out=w1a[:, :], in_=W1aug[0:P, :])
            nc.sync.dma_start(out=w1b[:, :], in_=W1aug[P:2 * P, :])
            w2_sb = cpool.tile([P, OUT_DIM + 2], BF16, tag="w2")
            nc.sync.dma_start(out=w2_sb[:, :], in_=W2aug[:, :])
            b1_sb = cpool.tile([P, HIDDEN], F32, tag="b1")
            nc.sync.dma_start(out=b1_sb[:, :], in_=b1F[:, :])
            b2_sb = cpool.tile([P, OUT_DIM], F32, tag="b2")
            nc.sync.dma_start(out=b2_sb[:, :], in_=b2F[:, :])
            iota_sb = cpool.tile([P, P], BF16, tag="iota")
            nc.sync.dma_start(out=iota_sb[:, :], in_=iotaF[:, :])
            iotaP_sb = cpool.tile([P, 1], F32, tag="iotaP")
            nc.sync.dma_start(out=iotaP_sb[:, :], in_=iotaP[:, :])
            ones_sb = cpool.tile([1, P], BF16, tag="ones1")
            nc.sync.dma_start(out=ones_sb[:, :], in_=ones1[:, :])
            id_sb = cpool.tile([P, P], BF16, tag="ident")
            nc.sync.dma_start(out=id_sb[:, :], in_=identT[:, :])

            # ---------------- Phase A: table1 = [x@W1aug] for all nodes
            SLAB = 512
            with (
                tc.tile_pool(name="pa_sbuf", bufs=3) as pa,
                tc.tile_pool(name="pa_lhs", bufs=2) as pl,
                tc.tile_pool(name="pa_psum", bufs=4, space="PSUM") as pp,
            ):
                for s in range(npad // SLAB):
                    lhs0 = pl.tile([P, SLAB], BF16, tag="lhs0")
                    lhs1 = pl.tile([P, SLAB], BF16, tag="lhs1")
                    nc.sync.dma_start(out=lhs0[:, :],
                                      in_=xT[0:P, s * SLAB:(s + 1) * SLAB])
                    nc.sync.dma_start(out=lhs1[:, :],
                                      in_=xT[P:2 * P, s * SLAB:(s + 1) * SLAB])
                    for ci in range(SLAB // P):
                        rbase = s * SLAB + ci * P
                        ps = pp.tile([P, C1 + 4], F32, tag="pa_ps", space="PSUM")
                        nc.tensor.matmul(ps[:, :],
                                         lhsT=lhs0[:, ci * P:(ci + 1) * P],
                                         rhs=w1a[:, :], start=True, stop=False)
                        nc.tensor.matmul(ps[:, :],
                                         lhsT=lhs1[:, ci * P:(ci + 1) * P],
                                         rhs=w1b[:, :], start=False, stop=True)
                        stage = pa.tile([P, C1 + 4], BF16, tag="pa_stage")
                        if ci % 2 == 0:
                            nc.vector.tensor_copy(out=stage[:, :], in_=ps[:, :])
                        else:
                            nc.scalar.copy(out=stage[:, :], in_=ps[:, :])
                        nc.sync.dma_start(
                            out=table1[rbase:rbase + P, 0:C1 + 4],
                            in_=stage[:, :])

            if stop_after != "A":
                _gat_layer(
                    nc, tc, layer=1, meta_units=meta["mu1"], n_tiles=n_tiles,
                    last_rows=last_rows, table=table1, tab_step=T1S,
                    qsize=qs1, hdim=C1, heads=HEADS, gcols=G1,
                    sidx=sidx1, dlocP=dlocP1, dlocF=dlocF1,
                    pcols=meta["pcols1"], own_tab=table1, iota_sb=iota_sb, iotaP_sb=iotaP_sb, ones_sb=ones_sb,
                    id_sb=id_sb, w2_sb=w2_sb,
                    b1_sb=b1_sb, b2_sb=b2_sb, adtab=table1, ad_step=T1S,
                    ad_lo=C1 + 2, t2shard=t2shard, out_shard=out_shard,
                    debug_out=(stop_after == "B"),
                )

            if stop_after is None:
                nc.gpsimd.collective_compute(
                    "AllGather", mybir.AluOpType.bypass,
                    replica_groups=[list(range(n_cores))],
                    ins=[t2shard[:, :]],
                    outs=[t2full[:, :]],
                )
                _gat_layer(
                    nc, tc, layer=2, meta_units=meta["mu2"], n_tiles=n_tiles,
                    last_rows=last_rows, table=t2full, tab_step=T2S,
                    qsize=qs2, hdim=OUT_DIM, heads=1, gcols=T2C,
                    sidx=sidx2, dlocP=dlocP2, dlocF=dlocF2,
                    pcols=meta["pcols2"], own_tab=t2shard, iota_sb=iota_sb, iotaP_sb=iotaP_sb, ones_sb=ones_sb,
                    id_sb=id_sb, w2_sb=w2_sb,
                    b1_sb=b1_sb, b2_sb=b2_sb, adtab=t2shard, ad_step=T2S,
                    ad_lo=OUT_DIM + 1, t2shard=t2shard, out_shard=out_shard,
                )
            elif stop_after == "A":
                with tc.tile_pool(name="dbgA", bufs=2) as pd:
                    for t in range(min(shard, 1024) // P):
                        st = pd.tile([P, C1 + 4], BF16, tag="dbgA_t")
                        nc.sync.dma_start(out=st[:, :],
                                          in_=table1[t * P:(t + 1) * P, 0:C1 + 4])
                        sf = pd.tile([P, OUT_DIM], F32, tag="dbgA_f")
                        nc.vector.tensor_copy(out=sf[:, :], in_=st[:, 0:OUT_DIM])
                        nc.sync.dma_start(out=out_shard[t * P:(t + 1) * P, :],
                                          in_=sf[0:P, :])

    nc.finalize()
    return nc


def _gat_layer(nc, tc, *, layer, meta_units, n_tiles, last_rows, table,
               tab_step, qsize, hdim, heads, gcols, sidx, dlocP, dlocF, pcols,
               own_tab, iota_sb, iotaP_sb, ones_sb, id_sb, w2_sb, b1_sb, b2_sb, adtab,
               ad_step, ad_lo, t2shard, out_shard, debug_out=False):
    as_off = hdim            # a_s column(s) in the gathered row
    rcols = hdim + heads     # matmul rhs cols (h plus per-head w)
    name = f"L{layer}"
    scol = 0                 # running idx-stream column (over /16 snake cols)
    ucount = 0               # running unit counter (global in layer)

    # adsb: a_d for all own-shard dsts: [128, n_tiles, heads]
    shard = (n_tiles - 1) * P + last_rows
    n_full = shard // P
    rem = shard - n_full * P
    with tc.tile_pool(name=f"{name}_ad", bufs=1) as pad:
        adsb = pad.tile([P, n_tiles, heads], BF16, tag="adsb")
        nc.vector.memset(adsb[:, :, :], 0.0)
        base = adtab[:, :]
        # rows t*128+p of adtab, cols ad_lo:ad_lo+heads (bounds-safe: only
        # the shard's real rows are read; the last partial tile via a 2nd DMA)
        if n_full:
            src_ap = bass.AP(
                base.tensor, base.offset + ad_lo,
                [[ad_step, P], [ad_step * P, n_full], [1, heads]])
            nc.sync.dma_start(out=adsb[:, 0:n_full, :], in_=src_ap)
        if rem:
            src2 = bass.AP(
                base.tensor, base.offset + ad_lo + ad_step * P * n_full,
                [[ad_step, rem], [1, heads]])
            nc.sync.dma_start(out=adsb[0:rem, n_full, :], in_=src2)

        with (
            tc.tile_pool(name=f"{name}_gath", bufs=3) as pg,
            tc.tile_pool(name=f"{name}_m", bufs=2) as pm,
            tc.tile_pool(name=f"{name}_mt", bufs=3) as pmt,
            tc.tile_pool(name=f"{name}_sm", bufs=3) as psm,
            tc.tile_pool(name=f"{name}_idx", bufs=3) as pidx,
            tc.tile_pool(name=f"{name}_psA", bufs=1, space="PSUM") as ppsA,
            tc.tile_pool(name=f"{name}_psB", bufs=2, space="PSUM") as ppsB,
            tc.tile_pool(name=f"{name}_psT", bufs=1, space="PSUM") as ppsT,
            tc.tile_pool(name=f"{name}_psE", bufs=1, space="PSUM") as ppsE,
        ):
            qidx = 0            # global quarter counter (dlocF row)
            for g_units in meta_units:
                g0, gn = g_units[0]["g0"], g_units[0]["gn"]
                # per-tile aggregation psums, live across the group
                aggs = [ppsA.tile([P, rcols], F32, tag=f"agg{ti}",
                                  name=f"agg{ti}", space="PSUM")
                        for ti in range(gn)]
                # first/last unit index per tile within this group
                tile_first, tile_last = {}, {}
                ug = 0
                for qd in g_units:
                    for (k, ti) in qd["units"]:
                        if ti not in tile_first:
                            tile_first[ti] = ug
                        tile_last[ti] = ug
                        ug += 1

                ug = 0
                for qd in g_units:
                    K, units, q = qd["K"], qd["units"], qd["q"]
                    ni = K * P
                    nu = len(units)
                    sid = pidx.tile([P, ni // 16], I16, tag="sidx")
                    nc.sync.dma_start(out=sid[:, :],
                                      in_=sidx[:, scol:scol + ni // 16])
                    scol += ni // 16

                    # gather [h | a_s] rows for the slab
                    hg = pg.tile([P, K, gcols], BF16, tag="hg")
                    _dma_gather(
                        nc.gpsimd, hg[:, :, :],
                        table[q * qsize:(q + 1) * qsize, 0:gcols],
                        sid[:, :], ni, gcols, tab_step)

                    # dloc streams for this quarter's units
                    dP = pidx.tile([P, nu], BF16, tag="dlocP")
                    nc.sync.dma_start(out=dP[:, :],
                                      in_=dlocP[:, ucount:ucount + nu])
                    dF = pidx.tile([1, nu * P], BF16, tag="dlocF")
                    fb = (qidx // 128) * pcols
                    nc.sync.dma_start(
                        out=dF[:, :],
                        in_=dlocF[qidx % 128:qidx % 128 + 1, fb:fb + nu * P])

                    # ---- slotB broadcast + mtT (windows of 4 units)
                    mtT = pmt.tile([P, nu, P], BF16, tag="mtT")
                    ub = 0
                    while ub < nu:
                        take = min(4, nu - ub)
                        ww = take * P
                        sB = ppsB.tile([P, 512], F32, tag="sB", space="PSUM")
                        nc.tensor.matmul(
                            sB[:, 0:ww], lhsT=ones_sb[:, :],
                            rhs=dF[0:1, ub * P:ub * P + ww],
                            start=True, stop=True)
                        mtT_v = mtT[:, ub:ub + take, :].rearrange(
                            "p a b -> p (a b)")
                        iP = iotaP_sb[:, :]
                        iota_b = bass.AP(iP.tensor, iP.offset,
                                         [list(iP.ap[0]), [0, ww]])
                        nc.vector.tensor_tensor(
                            out=mtT_v, in0=sB[:, 0:ww], in1=iota_b,
                            op=mybir.AluOpType.is_equal)
                        ub += take
                    qidx += 1

                    # ---- adE per chunk (accumulate units of same chunk)
                    adE = ppsE.tile([P, K, heads], F32, tag="adE",
                                    space="PSUM")
                    for uj, (k, ti) in enumerate(units):
                        first = (uj == 0) or units[uj - 1][0] != k
                        last = (uj == nu - 1) or units[uj + 1][0] != k
                        nc.tensor.matmul(
                            adE[:, k, :], lhsT=mtT[:, uj, :],
                            rhs=adsb[:, g0 + ti, :],
                            start=first, stop=last)

                    # ---- w = exp(leaky_relu(a_s + adE)); overwrite a_s cols
                    sE = psm.tile([P, K, heads], F32, tag="sE")
                    nc.vector.tensor_tensor(
                        out=sE[:, :, :], in0=hg[:, :, as_off:as_off + heads],
                        in1=adE[:, :, :], op=mybir.AluOpType.add)
                    lrE = psm.tile([P, K, heads], F32, tag="lrE")
                    nc.vector.scalar_tensor_tensor(
                        out=lrE[:, :, :], in0=sE[:, :, :], scalar=NEG_SLOPE,
                        in1=sE[:, :, :], op0=mybir.AluOpType.mult,
                        op1=mybir.AluOpType.max)
                    nc.scalar.activation(
                        out=hg[:, :, as_off:as_off + heads], in_=lrE[:, :, :],
                        func=mybir.ActivationFunctionType.Exp)

                    # ---- scale h rows by w (per head)
                    hv = hg[:, :, 0:hdim].rearrange(
                        "p a (h c) -> p a h c", h=heads)
                    wv = hg[:, :, as_off:as_off + heads].to_broadcast(
                        [P, K, heads, hdim // heads])
                    nc.vector.tensor_tensor(out=hv, in0=hv, in1=wv,
                                            op=mybir.AluOpType.mult)

                    # ---- mt (edge-major onehot) for all units of the quarter
                    mt = pm.tile([P, nu, P], BF16, tag="mt")
                    dP_v = dP[:, :].to_broadcast([P, nu, P])
                    io = iota_sb[:, :]
                    iota_v = bass.AP(io.tensor, io.offset,
                                     [list(io.ap[0]), [0, nu], [1, P]])
                    nc.vector.tensor_tensor(
                        out=mt[:, :, :], in0=dP_v, in1=iota_v,
                        op=mybir.AluOpType.is_equal)

                    # ---- aggregation matmuls
                    for uj, (k, ti) in enumerate(units):
                        nc.tensor.matmul(
                            aggs[ti][:, :], lhsT=mt[:, uj, :],
                            rhs=hg[:, k, 0:rcols],
                            start=(tile_first[ti] == ug),
                            stop=(tile_last[ti] == ug))
                        ug += 1
                    ucount += nu

                # ---- per-tile epilogue (self-loop merged densely)
                for ti in range(gn):
                    tid = g0 + ti
                    rows = last_rows if tid == n_tiles - 1 else P
                    ps = aggs[ti]

                    own = psm.tile([P, gcols], BF16, tag="own")
                    nc.sync.dma_start(
                        out=own[0:rows, :],
                        in_=own_tab[tid * P:tid * P + rows, 0:gcols])
                    sS = psm.tile([P, heads], F32, tag="sS")
                    nc.vector.tensor_tensor(
                        out=sS[0:rows, :], in0=own[0:rows, hdim:hdim + heads],
                        in1=adsb[0:rows, tid, :], op=mybir.AluOpType.add)
                    lS = psm.tile([P, heads], F32, tag="lS")
                    nc.vector.scalar_tensor_tensor(
                        out=lS[0:rows, :], in0=sS[0:rows, :], scalar=NEG_SLOPE,
                        in1=sS[0:rows, :], op0=mybir.AluOpType.mult,
                        op1=mybir.AluOpType.max)
                    wS = psm.tile([P, heads], F32, tag="wS")
                    nc.scalar.activation(
                        out=wS[0:rows, :], in_=lS[0:rows, :],
                        func=mybir.ActivationFunctionType.Exp)
                    hd = hdim // heads
                    for h in range(heads):
                        nc.vector.scalar_tensor_tensor(
                            out=ps[0:rows, h * hd:(h + 1) * hd],
                            in0=own[0:rows, h * hd:(h + 1) * hd],
                            scalar=wS[0:rows, h:h + 1],
                            in1=ps[0:rows, h * hd:(h + 1) * hd],
                            op0=mybir.AluOpType.mult, op1=mybir.AluOpType.add)
                    nc.vector.tensor_tensor(
                        out=ps[0:rows, hdim:hdim + heads],
                        in0=ps[0:rows, hdim:hdim + heads], in1=wS[0:rows, :],
                        op=mybir.AluOpType.add)

                    rec = psm.tile([P, heads], F32, tag="rec")
                    nc.vector.reciprocal(rec[:, :], ps[:, hdim:hdim + heads])

                    if layer == 1:
                        t0 = psm.tile([P, HIDDEN], F32, tag="t0")
                        nc.vector.tensor_scalar(
                            out=t0[:, :], in0=ps[:, 0:HIDDEN],
                            scalar1=rec[:, 0:1], scalar2=None,
                            op0=mybir.AluOpType.mult)
                        nc.vector.scalar_tensor_tensor(
                            out=t0[:, :], in0=ps[:, HIDDEN:2 * HIDDEN],
                            scalar=rec[:, 1:2], in1=t0[:, :],
                            op0=mybir.AluOpType.mult, op1=mybir.AluOpType.add)
                        hb = psm.tile([P, HIDDEN], F32, tag="hb")
                        nc.vector.scalar_tensor_tensor(
                            out=hb[:, :], in0=t0[:, :], scalar=0.5,
                            in1=b1_sb[:, :],
                            op0=mybir.AluOpType.mult, op1=mybir.AluOpType.add)
                        hr = psm.tile([P, HIDDEN], BF16, tag="hr")
                        nc.scalar.activation(
                            out=hr[:, :], in_=hb[:, :],
                            func=mybir.ActivationFunctionType.Relu)
                        psT = ppsT.tile([P, P], BF16, tag="psT", space="PSUM")
                        nc.tensor.transpose(out=psT[:, :], in_=hr[:, :],
                                            identity=id_sb[:, :])
                        hrT = psm.tile([P, P], BF16, tag="hrT")
                        nc.scalar.copy(out=hrT[:, :], in_=psT[:, :])
                        ps2 = ppsE.tile([P, OUT_DIM + 2], F32, tag="ps2",
                                        space="PSUM")
                        nc.tensor.matmul(ps2[:, :], lhsT=hrT[:, :],
                                         rhs=w2_sb[:, :],
                                         start=True, stop=True)
                        t2 = psm.tile([P, OUT_DIM + 2], BF16, tag="t2")
                        nc.vector.tensor_copy(out=t2[:, :], in_=ps2[:, :])
                        nc.sync.dma_start(
                            out=t2shard[tid * P:tid * P + rows, 0:OUT_DIM + 2],
                            in_=t2[0:rows, :])
                        if debug_out:
                            dbg = psm.tile([P, OUT_DIM], F32, tag="dbg")
                            nc.vector.tensor_copy(out=dbg[:, :],
                                                  in_=ps2[:, 0:OUT_DIM])
                            nc.sync.dma_start(
                                out=out_shard[tid * P:tid * P + rows, :],
                                in_=dbg[0:rows, :])
                    else:
                        of = psm.tile([P, OUT_DIM], F32, tag="of")
                        nc.vector.scalar_tensor_tensor(
                            out=of[:, :], in0=ps[:, 0:OUT_DIM],
                            scalar=rec[:, 0:1],
                            in1=b2_sb[:, :], op0=mybir.AluOpType.mult,
                            op1=mybir.AluOpType.add)
                        nc.sync.dma_start(
                            out=out_shard[tid * P:tid * P + rows, :],
                            in_=of[0:rows, :])


# ================================================================ entry point
def kernel(**inputs):
    cfg = dict(FULL_CFG)
    cfg["n"] = N
    in_maps, meta = _host_inputs(
        inputs["x"], inputs["edge_index"], inputs["W1"], inputs["att_src1"],
        inputs["att_dst1"], inputs["b1"], inputs["W2"], inputs["att_src2"],
        inputs["att_dst2"], inputs["b2"], cfg)
    nc = build_program(cfg, meta)
    # transient device wedges (NRT_EXEC_UNIT_UNRECOVERABLE) self-heal after a
    # few minutes; retry rather than failing the whole run
    import time as _time
    last = None
    for attempt in range(4):
        try:
            res = run_bass_kernel_spmd(
                nc, in_maps, core_ids=list(range(cfg["n_cores"])))
            break
        except Exception as exc:  # noqa: BLE001
            last = exc
            if attempt == 3:
                raise
            _time.sleep(90)
    out = np.concatenate(
        [res.results[c]["out_shard"] for c in range(cfg["n_cores"])], axis=0)
    return out.astype(np.float32)
